# revision 31
# baseline (speedup 1.0000x reference)
"""Trainium2 Bass kernel for CARE position encoding (rotor sandwich product).

out = R x R~ factorizes into 4 sequential Givens stages (planes e12,e03,e02,
e01 order): for plane bivector m, the 8 blades A with |A & m| == 1 rotate in
4 disjoint pairs (A, A^m) by angle 2*phi = pos*freq_i*coef_i with pair signs
tau = C[A, m, A^m]:
    out[a] = c*x[a] + tau*s*x[b] ;  out[b] = c*x[b] - tau*s*x[a]

v2 design (vs the position-major fp32 baseline at ~62us):
 - COMPONENT-MAJOR fp16 layout: per core the 2x16384 positions map to
   [128 partitions, 16 comps x J=256 positions], host pre-transposed.  Every
   DVE rotation op then has a dense step-1 innermost run of J elements in a
   2-byte dtype -> DVE 2x_1P mode (half the cycles of the fp32 baseline) and
   only 14 big tensor_tensor ops total instead of 40 small ones.
 - comps 0 and 15 (scalar/pseudoscalar) are invariant under the sandwich and
   never touch the device; host copies them through in fp32.
 - angle chain (A = pos*fc, magic-round k = round(A/2pi), R = A - 2pi*k) runs
   on the otherwise-idle GPSIMD/Pool engine in fp32; DVE only does the
   [-pi,pi] wrap for the cos stream; ScalarE turns R/RC into fp16 sin/cos
   tables (13 sign-slotted sin blocks + 4 cos blocks).
 - progressive output DMA: comps 7,8 leave after the e03 stage, 3,4,11,12
   after e02, the rest (j-split) after e01.
 - every plane's index/sign arithmetic is verified symbolically against the
   input Cayley tensor at kernel-build time.
"""
import numpy as np

import concourse.bass as bass
import concourse.tile as tile
from concourse import bacc, mybir
from concourse.bass_utils import run_bass_kernel_spmd

F32 = mybir.dt.float32
F16 = mybir.dt.float16
ALU = mybir.AluOpType

P = 128
NCORES = 8
B, L, MV = 16, 16384, 16
MAX_LEN = 16384
ROWS_PER_CORE = B // NCORES          # 2
N = ROWS_PER_CORE * L                # 32768 positions per core
J = N // P                           # 256 positions per partition
JH = J // 2



# stage application order (innermost rotor first): (coef idx, blade)
STAGES = ((3, 6), (2, 9), (1, 5), (0, 3))
PLANE_BLADES = (3, 5, 9, 6)

MAGIC = float(np.float32(1.5 * 2 ** 23))
TWO_PI = 2.0 * np.pi
INV_2PI = float(np.float32(1.0 / TWO_PI))
NEG_2PI = float(np.float32(-TWO_PI))
PI_F = float(np.float32(np.pi))
HALF_PI = float(np.float32(np.pi / 2.0))
TWO_PI_F = float(np.float32(TWO_PI))

# ---- per-plane rotation descriptors (comp-major layout) ----
# dims/offsets are in units of J elements (one component-column block).
# Each sub: xoff/xdims = component offset / [step,count] outer dims of the
# X-tile access; toff/tdims = matching slot offset/dims in the dense 8-slot
# T/U tiles.  usubs add soff/ssteps: sign-block index = soff + sum(ssteps*idx)
# into SBLK[m].  A [1, J] innermost dim is appended to everything at emit.
ROT = {
    6: dict(  # e12: pairs (8h+2+r, ^6); tau = +,+,-,- over r
        tsubs=[dict(xoff=2, xdims=[[8, 2], [1, 4]], toff=0, tdims=[[4, 2], [1, 4]])],
        usubs=[dict(xoff=4, xdims=[[8, 2], [1, 2]], toff=0, tdims=[[4, 2], [1, 2]],
                    soff=0, ssteps=[0, 0]),
               dict(xoff=2, xdims=[[8, 2], [1, 2]], toff=2, tdims=[[4, 2], [1, 2]],
                    soff=1, ssteps=[0, 0])],
        asubs=[dict(xoff=2, xdims=[[8, 2], [1, 4]], toff=0, tdims=[[4, 2], [1, 4]])],
    ),
    9: dict(  # e03: pairs (1+7w+2u, ^9); tau = (+,-,-,+) * (-1)^w
        tsubs=[dict(xoff=1, xdims=[[7, 2], [2, 4]], toff=0, tdims=[[4, 2], [1, 4]])],
        usubs=[dict(xoff=8, xdims=[[-7, 2], [2, 4]], toff=0, tdims=[[4, 2], [1, 4]],
                    soff=0, ssteps=[2, 1])],
        asubs=[dict(xoff=1, xdims=[[7, 2], [2, 4]], toff=0, tdims=[[4, 2], [1, 4]])],
    ),
    5: dict(  # e02: pairs (1+3d1+2d2+8h, ^5); tau = +,-,-,+ over (d1,d2)
        tsubs=[dict(xoff=1 + 8 * h, xdims=[[3, 2], [2, 2]],
                    toff=4 * h, tdims=[[2, 2], [1, 2]]) for h in (0, 1)],
        usubs=[dict(xoff=4 + 8 * h, xdims=[[-3, 2], [2, 2]],
                    toff=4 * h, tdims=[[2, 2], [1, 2]],
                    soff=0, ssteps=[1, 1]) for h in (0, 1)],
        asubs=[dict(xoff=1 + 8 * h, xdims=[[3, 2], [2, 2]],
                    toff=4 * h, tdims=[[2, 2], [1, 2]]) for h in (0, 1)],
    ),
    3: dict(  # e01: pairs (1+4q+r, ^3); tau = +,- over r
        tsubs=[dict(xoff=1, xdims=[[4, 4], [1, 2]], toff=0, tdims=[[2, 4], [1, 2]])],
        usubs=[dict(xoff=2, xdims=[[4, 4], [-1, 2]], toff=0, tdims=[[2, 4], [1, 2]],
                    soff=0, ssteps=[0, 1])],
        # split by comp pair so each finished pair ({1,2},{5,6},{9,10},
        # {13,14} - all comp-contiguous) drips out on its own DMA queue
        asubs=[dict(xoff=1 + 4 * q, xdims=[[1, 2]],
                    toff=2 * q, tdims=[[1, 2]]) for q in range(4)],
    ),
}

# sign-block sequences and bases (in J units) within the S table
SBLK = {6: (1, -1), 9: (1, -1, -1, 1, 1, -1), 5: (1, -1, 1), 3: (1, -1)}
SBASE = {6: 0, 9: 2, 5: 8, 3: 11}
S_TOTAL = 13
# ScalarE emission runs: (block_start, block_stride, count, sign)
S_RUNS = {
    6: [(0, 1, 1, 1.0), (1, 1, 1, -1.0)],
    9: [(0, 3, 2, 1.0), (4, 1, 1, 1.0), (1, 1, 2, -1.0), (5, 1, 1, -1.0)],
    5: [(0, 2, 2, 1.0), (1, 1, 1, -1.0)],
    3: [(0, 1, 1, 1.0), (1, 1, 1, -1.0)],
}


def _iter_idx(dims):
    import itertools
    return itertools.product(*[range(c) for (_, c) in dims])


def _verify_rot(cayley):
    """Symbolically expand the descriptor index arithmetic (one position):
    out[a] = c*x[tcomp] + blk_sign*s*x[ucomp] must equal the Cayley-derived
    Givens stage for every plane.  Raises on mismatch."""
    for m in PLANE_BLADES:
        ops = ROT[m]
        tmap, umap, usgn, amap = {}, {}, {}, {}
        for sub in ops["tsubs"]:
            for idx in _iter_idx(sub["xdims"]):
                slot = sub["toff"] + sum(s * i for (s, _), i in zip(sub["tdims"], idx))
                comp = sub["xoff"] + sum(s * i for (s, _), i in zip(sub["xdims"], idx))
                tmap[slot] = comp
        for sub in ops["usubs"]:
            for idx in _iter_idx(sub["xdims"]):
                slot = sub["toff"] + sum(s * i for (s, _), i in zip(sub["tdims"], idx))
                comp = sub["xoff"] + sum(s * i for (s, _), i in zip(sub["xdims"], idx))
                blk = sub["soff"] + sum(s * i for s, i in zip(sub["ssteps"], idx))
                assert 0 <= blk < len(SBLK[m]), (m, blk)
                umap[slot] = comp
                usgn[slot] = SBLK[m][blk]
        for sub in ops["asubs"]:
            for idx in _iter_idx(sub["xdims"]):
                slot = sub["toff"] + sum(s * i for (s, _), i in zip(sub["tdims"], idx))
                comp = sub["xoff"] + sum(s * i for (s, _), i in zip(sub["xdims"], idx))
                amap[slot] = comp
        assert sorted(tmap) == sorted(umap) == sorted(amap) == list(range(8)), m
        for slot in range(8):
            a = amap[slot]
            assert tmap[slot] == a, (m, slot, "cos part must read the dst comp")
            b = a ^ m
            assert umap[slot] == b, (m, slot, umap[slot], b)
            tau = float(cayley[a, m, b])
            assert usgn[slot] == tau, (m, slot, usgn[slot], tau)
        # S_RUNS must cover each block exactly once with the right sign
        cov = {}
        for b0, bs, cnt, sgn in S_RUNS[m]:
            for i in range(cnt):
                blk = b0 + bs * i
                assert blk not in cov, (m, blk)
                cov[blk] = sgn
        assert sorted(cov) == list(range(len(SBLK[m]))), m
        for blk, sgn in cov.items():
            assert sgn == SBLK[m][blk], (m, blk)


def _ap_with_dims(base_ap, extra_off, dims):
    ap = [list(base_ap.ap[0])] + [list(d) for d in dims]
    return bass.AP(base_ap.tensor, base_ap.offset + extra_off, ap)


def _build_program(freqs, coefs):
    nc = bacc.Bacc("TRN2", target_bir_lowering=False, debug=False,
                   enable_asserts=False, num_devices=NCORES)
    x_d = nc.dram_tensor("x", [P, MV * J], F16, kind="ExternalInput")
    posf_d = nc.dram_tensor("posf", [P, J], F32, kind="ExternalInput")
    out_d = nc.dram_tensor("out", [P, MV * J], F16, kind="ExternalOutput")

    SIN = mybir.ActivationFunctionType.Sin
    fc = [float(np.float32(np.float64(freqs[i]) * np.float64(coefs[i])))
          for i in range(4)]
    # planes whose full angle stays inside [-pi/2, pi/2] need no range
    # reduction at all: sin AND cos (= Sin(pi/2 - A)) come straight off
    # ScalarE from posf.  With the RoPE schedule this covers the two
    # low-frequency planes; the rest use the q-phase magic-round chain.
    direct = [abs(fc[i]) * (MAX_LEN - 1) < 1.5 for i in range(4)]

    with tile.TileContext(nc) as tc:
        with tc.tile_pool(name="io", bufs=1) as iop, \
             tc.tile_pool(name="ang", bufs=1) as ang, \
             tc.tile_pool(name="work", bufs=1) as wp:

            POSF = iop.tile([P, J], F32)
            X = iop.tile([P, MV * J], F16)
            nc.sync.dma_start(POSF[:], posf_d[:])
            # comps 1..14 (0 and 15 are invariant; handled host-side),
            # split across DMA queues so the rings transfer in parallel and
            # plane 0's components (2-5, 10-13) land first -- a single-queue
            # 896KB transfer (~6us) was gating the first rotation
            for lo, hi in ((2, 6), (10, 14), (6, 10), (1, 2), (14, 15)):
                nc.sync.dma_start(X[:, lo * J:hi * J], x_d[:, lo * J:hi * J])

            # q-phase angle pipeline, in units of turns: q = pos*fc/2pi and
            # (for the cos stream) q' = 0.25 - q.  F = round(q) - q so that
            # sin(angle) = Sin(-2pi*F) with the 2pi scale folded into the
            # ScalarE Sin activation; cos(angle) = sin(2pi*(0.25 - q)) via
            # the q' stream.  Per plane: columns [2i*J, (2i+1)*J) hold the
            # sin stream, [(2i+1)*J, (2i+2)*J) the cos stream.
            Q = ang.tile([P, 8 * J], F32)
            K1 = ang.tile([P, 8 * J], F32)
            F = ang.tile([P, 8 * J], F32)
            CT = ang.tile([P, 4 * J], F16)
            ST = ang.tile([P, S_TOTAL * J], F16)

            T = wp.tile([P, 8 * J], F16)
            U = wp.tile([P, 8 * J], F16)

            COPY = mybir.ActivationFunctionType.Copy
            fcq = [float(np.float32(np.float64(fc[i]) / TWO_PI))
                   for i in range(4)]

            # prime the ScalarE Sin lookup table while the posf DMA is still
            # in flight so the 1.3us ACT_TABLE_LOAD stays off the critical path
            SCR = ang.tile([P, 4], F32)
            nc.vector.memset(SCR[:], 0.0)
            nc.scalar.activation(SCR[:], SCR[:], SIN)
            # per-partition pi/2 bias operand for the direct-cos Sin ACTs
            # (Sin activations need a const AP, not an immediate bias)
            HPI = ang.tile([P, 1], F32)
            nc.vector.memset(HPI[:], HALF_PI)

            def affine_c(i):
                sc = slice((2 * i + 1) * J, (2 * i + 2) * J)
                nc.scalar.activation(Q[:, sc], POSF[:], COPY,
                                     scale=-fcq[i], bias=0.25)
                nc.scalar.activation(K1[:, sc], Q[:, sc], COPY, bias=MAGIC)

            def affine_s(i):
                ss = slice(2 * i * J, (2 * i + 1) * J)
                nc.scalar.activation(Q[:, ss], POSF[:], COPY, scale=fcq[i])
                nc.scalar.activation(K1[:, ss], Q[:, ss], COPY, bias=MAGIC)

            def fstep(i, lo, n, eng):
                # F = (K1 - MAGIC) - Q = round(q) - q, exact in fp32
                s2 = slice((2 * i + lo) * J, (2 * i + lo + n) * J)
                eng.scalar_tensor_tensor(F[:, s2], K1[:, s2], MAGIC,
                                         Q[:, s2], ALU.subtract,
                                         ALU.subtract)

            def cos_t(i):
                sl = slice(i * J, (i + 1) * J)
                nc.scalar.activation(CT[:, sl], F[:, (2 * i + 1) * J:
                                                    (2 * i + 2) * J],
                                     SIN, scale=NEG_2PI)

            def sin_t(i, m):
                for b0, bs, cnt, sgn in S_RUNS[m]:
                    dst = _ap_with_dims(ST[:], (SBASE[m] + b0) * J,
                                        [[bs * J, cnt], [1, J]])
                    src = _ap_with_dims(F[:], 2 * i * J, [[0, cnt], [1, J]])
                    nc.scalar.activation(dst, src, SIN,
                                         scale=NEG_2PI * sgn)

            def tables_direct(i, m):
                # no-range-reduction path: C and S straight from posf
                nc.scalar.activation(CT[:, i * J:(i + 1) * J], POSF[:], SIN,
                                     scale=-fc[i], bias=HPI[:])
                for b0, bs, cnt, sgn in S_RUNS[m]:
                    dst = _ap_with_dims(ST[:], (SBASE[m] + b0) * J,
                                        [[bs * J, cnt], [1, J]])
                    src = _ap_with_dims(POSF[:], 0, [[0, cnt], [1, J]])
                    nc.scalar.activation(dst, src, SIN, scale=fc[i] * sgn)

            # stage 0 and 1 tables up front (direct planes are pure ScalarE;
            # a non-direct stage here falls back to the q-phase chain with
            # its F-step placed in the DVE's pre-rotation idle window)
            for s in (0, 1):
                si, sm = STAGES[s]
                if direct[si]:
                    tables_direct(si, sm)
                else:
                    affine_c(si)
                    affine_s(si)
                    fstep(si, 0, 2, nc.vector)
                    cos_t(si)
                    sin_t(si, sm)

            for k, (i, m) in enumerate(STAGES):
                ops = ROT[m]
                for sub in ops["tsubs"]:
                    xr = _ap_with_dims(X[:], sub["xoff"] * J,
                                       [[s * J, c] for s, c in sub["xdims"]]
                                       + [[1, J]])
                    tw = _ap_with_dims(T[:], sub["toff"] * J,
                                       [[s * J, c] for s, c in sub["tdims"]]
                                       + [[1, J]])
                    cb = _ap_with_dims(CT[:], i * J,
                                       [[0, c] for (_, c) in sub["tdims"]]
                                       + [[1, J]])
                    nc.vector.tensor_mul(tw, xr, cb)
                for sub in ops["usubs"]:
                    xr = _ap_with_dims(X[:], sub["xoff"] * J,
                                       [[s * J, c] for s, c in sub["xdims"]]
                                       + [[1, J]])
                    uw = _ap_with_dims(U[:], sub["toff"] * J,
                                       [[s * J, c] for s, c in sub["tdims"]]
                                       + [[1, J]])
                    sb = _ap_with_dims(ST[:], (SBASE[m] + sub["soff"]) * J,
                                       [[s * J, c] for s, c in
                                        zip(sub["ssteps"],
                                            [c for (_, c) in sub["tdims"]])]
                                       + [[1, J]])
                    nc.vector.tensor_mul(uw, xr, sb)

                if k + 1 < len(STAGES) and k >= 1:
                    ni, nm = STAGES[k + 1]
                    if direct[ni]:
                        tables_direct(ni, nm)
                    else:
                        fstep(ni, 0, 2, nc.vector)
                        cos_t(ni)
                        sin_t(ni, nm)
                if k + 2 < len(STAGES) and not direct[STAGES[k + 2][0]]:
                    affine_c(STAGES[k + 2][0])
                    affine_s(STAGES[k + 2][0])

                final = k == len(STAGES) - 1
                for sub in ops["asubs"]:
                    xw = _ap_with_dims(X[:], sub["xoff"] * J,
                                       [[s * J, c] for s, c in sub["xdims"]]
                                       + [[1, J]])
                    tv = _ap_with_dims(T[:], sub["toff"] * J,
                                       [[s * J, c] for s, c in sub["tdims"]]
                                       + [[1, J]])
                    uv = _ap_with_dims(U[:], sub["toff"] * J,
                                       [[s * J, c] for s, c in sub["tdims"]]
                                       + [[1, J]])
                    nc.vector.tensor_add(xw, tv, uv)
                    if final:
                        # this comp pair is done: one contiguous 2J run
                        dsts = _ap_with_dims(out_d[:], sub["xoff"] * J,
                                             [[1, 2 * J]])
                        srcs = _ap_with_dims(X[:], sub["xoff"] * J,
                                             [[1, 2 * J]])
                        nc.sync.dma_start(dsts, srcs)

                if m == 9:   # comps 7,8 final
                    nc.sync.dma_start(out_d[:, 7 * J:9 * J], X[:, 7 * J:9 * J])
                if m == 5:   # comps 3,4,11,12 final (pairs comp-contiguous)
                    dsts = _ap_with_dims(out_d[:], 3 * J,
                                         [[8 * J, 2], [1, 2 * J]])
                    srcs = _ap_with_dims(X[:], 3 * J,
                                         [[8 * J, 2], [1, 2 * J]])
                    nc.sync.dma_start(dsts, srcs)

    nc.compile()
    return nc


_PROGRAM_CACHE = {}


def _get_program(freqs, coefs):
    key = (tuple(freqs), tuple(coefs))
    if key not in _PROGRAM_CACHE:
        _PROGRAM_CACHE[key] = _build_program(freqs, coefs)
    return _PROGRAM_CACHE[key]


def _core_inputs(x, pos_i, g):
    """Per-core comp-major fp16 x and fp32 posf."""
    xg = np.ascontiguousarray(
        x[g * ROWS_PER_CORE:(g + 1) * ROWS_PER_CORE]).reshape(P, J, MV)
    xg = np.ascontiguousarray(xg.transpose(0, 2, 1)).astype(np.float16)
    pg = pos_i[g * ROWS_PER_CORE:(g + 1) * ROWS_PER_CORE].reshape(P, J)
    return {"x": xg.reshape(P, MV * J),
            "posf": np.ascontiguousarray(pg).astype(np.float32)}


def kernel(x, pos, bx, by, bz, bw, theta, cayley, biv_mask, scalar_mask):
    x = np.asarray(x, dtype=np.float32)
    pos = np.asarray(pos)
    theta = np.asarray(theta, dtype=np.float32)
    cayley = np.asarray(cayley, dtype=np.float32)

    assert x.shape == (B, L, MV) and pos.shape == (B, L)

    coefs = [float(np.asarray(c, dtype=np.float32).reshape(MV)[b])
             for c, b in zip((bx, by, bz, bw), PLANE_BLADES)]
    freqs = [float(theta.reshape(MAX_LEN, 4)[1, i]) for i in range(4)]
    th_check = np.arange(MAX_LEN, dtype=np.float32)[:, None] * \
        np.asarray(freqs, dtype=np.float32)[None, :]
    assert np.array_equal(th_check, theta.reshape(MAX_LEN, 4)), \
        "theta table is not linear in position; kernel assumption violated"

    _verify_rot(cayley)

    nc = _get_program(freqs, coefs)

    pos_i = np.clip(pos, 0, MAX_LEN - 1).astype(np.int32)
    in_maps = [_core_inputs(x, pos_i, g) for g in range(NCORES)]

    res = run_bass_kernel_spmd(nc, in_maps, core_ids=list(range(NCORES)))
    out = np.empty((B, L, MV), dtype=np.float32)
    for g in range(NCORES):
        og = res.results[g]["out"].reshape(P, MV, J).astype(np.float32)
        out[g * ROWS_PER_CORE:(g + 1) * ROWS_PER_CORE] = \
            og.transpose(0, 2, 1).reshape(ROWS_PER_CORE, L, MV)
    # scalar / pseudoscalar components are invariant under the sandwich
    out[..., 0] = x[..., 0]
    out[..., 15] = x[..., 15]
    return out


# revision 33
# speedup vs baseline: 1.0280x; 1.0280x over previous
"""Trainium2 Bass kernel for CARE position encoding (rotor sandwich product).

out = R x R~ factorizes into 4 sequential Givens stages (planes e12,e03,e02,
e01 order): for plane bivector m, the 8 blades A with |A & m| == 1 rotate in
4 disjoint pairs (A, A^m) by angle 2*phi = pos*freq_i*coef_i with pair signs
tau = C[A, m, A^m]:
    out[a] = c*x[a] + tau*s*x[b] ;  out[b] = c*x[b] - tau*s*x[a]

v2 design (vs the position-major fp32 baseline at ~62us):
 - COMPONENT-MAJOR fp16 layout: per core the 2x16384 positions map to
   [128 partitions, 16 comps x J=256 positions], host pre-transposed.  Every
   DVE rotation op then has a dense step-1 innermost run of J elements in a
   2-byte dtype -> DVE 2x_1P mode (half the cycles of the fp32 baseline) and
   only 14 big tensor_tensor ops total instead of 40 small ones.
 - comps 0 and 15 (scalar/pseudoscalar) are invariant under the sandwich and
   never touch the device; host copies them through in fp32.
 - angle chain (A = pos*fc, magic-round k = round(A/2pi), R = A - 2pi*k) runs
   on the otherwise-idle GPSIMD/Pool engine in fp32; DVE only does the
   [-pi,pi] wrap for the cos stream; ScalarE turns R/RC into fp16 sin/cos
   tables (13 sign-slotted sin blocks + 4 cos blocks).
 - progressive output DMA: comps 7,8 leave after the e03 stage, 3,4,11,12
   after e02, the rest (j-split) after e01.
 - every plane's index/sign arithmetic is verified symbolically against the
   input Cayley tensor at kernel-build time.
"""
import numpy as np

import concourse.bass as bass
import concourse.tile as tile
from concourse import bacc, mybir
from concourse.bass_utils import run_bass_kernel_spmd

F32 = mybir.dt.float32
F16 = mybir.dt.float16
ALU = mybir.AluOpType

P = 128
NCORES = 8
B, L, MV = 16, 16384, 16
MAX_LEN = 16384
ROWS_PER_CORE = B // NCORES          # 2
N = ROWS_PER_CORE * L                # 32768 positions per core
J = N // P                           # 256 positions per partition
JH = J // 2



# stage application order (innermost rotor first): (coef idx, blade)
STAGES = ((3, 6), (2, 9), (1, 5), (0, 3))
PLANE_BLADES = (3, 5, 9, 6)

MAGIC = float(np.float32(1.5 * 2 ** 23))
TWO_PI = 2.0 * np.pi
INV_2PI = float(np.float32(1.0 / TWO_PI))
NEG_2PI = float(np.float32(-TWO_PI))
PI_F = float(np.float32(np.pi))
HALF_PI = float(np.float32(np.pi / 2.0))
TWO_PI_F = float(np.float32(TWO_PI))

# ---- per-plane rotation descriptors (comp-major layout) ----
# dims/offsets are in units of J elements (one component-column block).
# Each sub: xoff/xdims = component offset / [step,count] outer dims of the
# X-tile access; toff/tdims = matching slot offset/dims in the dense 8-slot
# T/U tiles.  usubs add soff/ssteps: sign-block index = soff + sum(ssteps*idx)
# into SBLK[m].  A [1, J] innermost dim is appended to everything at emit.
ROT = {
    6: dict(  # e12: pairs (8h+2+r, ^6); tau = +,+,-,- over r
        tsubs=[dict(xoff=2, xdims=[[8, 2], [1, 4]], toff=0, tdims=[[4, 2], [1, 4]])],
        usubs=[dict(xoff=4, xdims=[[8, 2], [1, 2]], toff=0, tdims=[[4, 2], [1, 2]],
                    soff=0, ssteps=[0, 0]),
               dict(xoff=2, xdims=[[8, 2], [1, 2]], toff=2, tdims=[[4, 2], [1, 2]],
                    soff=1, ssteps=[0, 0])],
        asubs=[dict(xoff=2, xdims=[[8, 2], [1, 4]], toff=0, tdims=[[4, 2], [1, 4]])],
    ),
    9: dict(  # e03: pairs (1+7w+2u, ^9); tau = (+,-,-,+) * (-1)^w
        tsubs=[dict(xoff=1, xdims=[[7, 2], [2, 4]], toff=0, tdims=[[4, 2], [1, 4]])],
        usubs=[dict(xoff=8, xdims=[[-7, 2], [2, 4]], toff=0, tdims=[[4, 2], [1, 4]],
                    soff=0, ssteps=[2, 1])],
        asubs=[dict(xoff=1, xdims=[[7, 2], [2, 4]], toff=0, tdims=[[4, 2], [1, 4]])],
    ),
    5: dict(  # e02: pairs (1+3d1+2d2+8h, ^5); tau = +,-,-,+ over (d1,d2)
        tsubs=[dict(xoff=1 + 8 * h, xdims=[[3, 2], [2, 2]],
                    toff=4 * h, tdims=[[2, 2], [1, 2]]) for h in (0, 1)],
        usubs=[dict(xoff=4 + 8 * h, xdims=[[-3, 2], [2, 2]],
                    toff=4 * h, tdims=[[2, 2], [1, 2]],
                    soff=0, ssteps=[1, 1]) for h in (0, 1)],
        asubs=[dict(xoff=1 + 8 * h, xdims=[[3, 2], [2, 2]],
                    toff=4 * h, tdims=[[2, 2], [1, 2]]) for h in (0, 1)],
    ),
    3: dict(  # e01: pairs (1+4q+r, ^3); tau = +,- over r
        tsubs=[dict(xoff=1, xdims=[[4, 4], [1, 2]], toff=0, tdims=[[2, 4], [1, 2]])],
        usubs=[dict(xoff=2, xdims=[[4, 4], [-1, 2]], toff=0, tdims=[[2, 4], [1, 2]],
                    soff=0, ssteps=[0, 1])],
        # split by comp quartet ({1,2,5,6} then {9,10,13,14}) - finer
        # pair-level drip loses more to serial DMA descriptor-gen (~0.65us
        # each on the sync queue) than it gains in transfer overlap
        asubs=[dict(xoff=1 + 8 * h, xdims=[[4, 2], [1, 2]],
                    toff=4 * h, tdims=[[2, 2], [1, 2]]) for h in (0, 1)],
    ),
}

# sign-block sequences and bases (in J units) within the S table
SBLK = {6: (1, -1), 9: (1, -1, -1, 1, 1, -1), 5: (1, -1, 1), 3: (1, -1)}
SBASE = {6: 0, 9: 2, 5: 8, 3: 11}
S_TOTAL = 13
# ScalarE emission runs: (block_start, block_stride, count, sign)
S_RUNS = {
    6: [(0, 1, 1, 1.0), (1, 1, 1, -1.0)],
    9: [(0, 3, 2, 1.0), (4, 1, 1, 1.0), (1, 1, 2, -1.0), (5, 1, 1, -1.0)],
    5: [(0, 2, 2, 1.0), (1, 1, 1, -1.0)],
    3: [(0, 1, 1, 1.0), (1, 1, 1, -1.0)],
}


def _iter_idx(dims):
    import itertools
    return itertools.product(*[range(c) for (_, c) in dims])


def _verify_rot(cayley):
    """Symbolically expand the descriptor index arithmetic (one position):
    out[a] = c*x[tcomp] + blk_sign*s*x[ucomp] must equal the Cayley-derived
    Givens stage for every plane.  Raises on mismatch."""
    for m in PLANE_BLADES:
        ops = ROT[m]
        tmap, umap, usgn, amap = {}, {}, {}, {}
        for sub in ops["tsubs"]:
            for idx in _iter_idx(sub["xdims"]):
                slot = sub["toff"] + sum(s * i for (s, _), i in zip(sub["tdims"], idx))
                comp = sub["xoff"] + sum(s * i for (s, _), i in zip(sub["xdims"], idx))
                tmap[slot] = comp
        for sub in ops["usubs"]:
            for idx in _iter_idx(sub["xdims"]):
                slot = sub["toff"] + sum(s * i for (s, _), i in zip(sub["tdims"], idx))
                comp = sub["xoff"] + sum(s * i for (s, _), i in zip(sub["xdims"], idx))
                blk = sub["soff"] + sum(s * i for s, i in zip(sub["ssteps"], idx))
                assert 0 <= blk < len(SBLK[m]), (m, blk)
                umap[slot] = comp
                usgn[slot] = SBLK[m][blk]
        for sub in ops["asubs"]:
            for idx in _iter_idx(sub["xdims"]):
                slot = sub["toff"] + sum(s * i for (s, _), i in zip(sub["tdims"], idx))
                comp = sub["xoff"] + sum(s * i for (s, _), i in zip(sub["xdims"], idx))
                amap[slot] = comp
        assert sorted(tmap) == sorted(umap) == sorted(amap) == list(range(8)), m
        for slot in range(8):
            a = amap[slot]
            assert tmap[slot] == a, (m, slot, "cos part must read the dst comp")
            b = a ^ m
            assert umap[slot] == b, (m, slot, umap[slot], b)
            tau = float(cayley[a, m, b])
            assert usgn[slot] == tau, (m, slot, usgn[slot], tau)
        # S_RUNS must cover each block exactly once with the right sign
        cov = {}
        for b0, bs, cnt, sgn in S_RUNS[m]:
            for i in range(cnt):
                blk = b0 + bs * i
                assert blk not in cov, (m, blk)
                cov[blk] = sgn
        assert sorted(cov) == list(range(len(SBLK[m]))), m
        for blk, sgn in cov.items():
            assert sgn == SBLK[m][blk], (m, blk)


def _ap_with_dims(base_ap, extra_off, dims):
    ap = [list(base_ap.ap[0])] + [list(d) for d in dims]
    return bass.AP(base_ap.tensor, base_ap.offset + extra_off, ap)


def _build_program(freqs, coefs):
    nc = bacc.Bacc("TRN2", target_bir_lowering=False, debug=False,
                   enable_asserts=False, num_devices=NCORES)
    x_d = nc.dram_tensor("x", [P, MV * J], F16, kind="ExternalInput")
    posf_d = nc.dram_tensor("posf", [P, J], F32, kind="ExternalInput")
    out_d = nc.dram_tensor("out", [P, MV * J], F16, kind="ExternalOutput")

    SIN = mybir.ActivationFunctionType.Sin
    fc = [float(np.float32(np.float64(freqs[i]) * np.float64(coefs[i])))
          for i in range(4)]
    # planes whose full angle stays inside [-pi/2, pi/2] need no range
    # reduction at all: sin AND cos (= Sin(pi/2 - A)) come straight off
    # ScalarE from posf.  With the RoPE schedule this covers the two
    # low-frequency planes; the rest use the q-phase magic-round chain.
    direct = [abs(fc[i]) * (MAX_LEN - 1) < 1.5 for i in range(4)]

    with tile.TileContext(nc) as tc:
        with tc.tile_pool(name="io", bufs=1) as iop, \
             tc.tile_pool(name="ang", bufs=1) as ang, \
             tc.tile_pool(name="work", bufs=1) as wp:

            POSF = iop.tile([P, J], F32)
            X = iop.tile([P, MV * J], F16)
            nc.sync.dma_start(POSF[:], posf_d[:])
            # comps 1..14 (0 and 15 are invariant; handled host-side),
            # split across DMA queues so the rings transfer in parallel and
            # plane 0's components (2-5, 10-13) land first -- a single-queue
            # 896KB transfer (~6us) was gating the first rotation
            for lo, hi in ((2, 6), (10, 14), (6, 10), (1, 2), (14, 15)):
                nc.sync.dma_start(X[:, lo * J:hi * J], x_d[:, lo * J:hi * J])

            # q-phase angle pipeline, in units of turns: q = pos*fc/2pi and
            # (for the cos stream) q' = 0.25 - q.  F = round(q) - q so that
            # sin(angle) = Sin(-2pi*F) with the 2pi scale folded into the
            # ScalarE Sin activation; cos(angle) = sin(2pi*(0.25 - q)) via
            # the q' stream.  Per plane: columns [2i*J, (2i+1)*J) hold the
            # sin stream, [(2i+1)*J, (2i+2)*J) the cos stream.
            Q = ang.tile([P, 8 * J], F32)
            K1 = ang.tile([P, 8 * J], F32)
            F = ang.tile([P, 8 * J], F32)
            CT = ang.tile([P, 4 * J], F16)
            ST = ang.tile([P, S_TOTAL * J], F16)

            T = wp.tile([P, 8 * J], F16)
            U = wp.tile([P, 8 * J], F16)

            COPY = mybir.ActivationFunctionType.Copy
            fcq = [float(np.float32(np.float64(fc[i]) / TWO_PI))
                   for i in range(4)]

            # prime the ScalarE Sin lookup table while the posf DMA is still
            # in flight so the 1.3us ACT_TABLE_LOAD stays off the critical path
            SCR = ang.tile([P, 4], F32)
            nc.vector.memset(SCR[:], 0.0)
            nc.scalar.activation(SCR[:], SCR[:], SIN)
            # per-partition pi/2 bias operand for the direct-cos Sin ACTs
            # (Sin activations need a const AP, not an immediate bias)
            HPI = ang.tile([P, 1], F32)
            nc.vector.memset(HPI[:], HALF_PI)

            def affine_c(i):
                sc = slice((2 * i + 1) * J, (2 * i + 2) * J)
                nc.scalar.activation(Q[:, sc], POSF[:], COPY,
                                     scale=-fcq[i], bias=0.25)
                nc.scalar.activation(K1[:, sc], Q[:, sc], COPY, bias=MAGIC)

            def affine_s(i):
                ss = slice(2 * i * J, (2 * i + 1) * J)
                nc.scalar.activation(Q[:, ss], POSF[:], COPY, scale=fcq[i])
                nc.scalar.activation(K1[:, ss], Q[:, ss], COPY, bias=MAGIC)

            def fstep(i, lo, n, eng):
                # F = (K1 - MAGIC) - Q = round(q) - q, exact in fp32
                s2 = slice((2 * i + lo) * J, (2 * i + lo + n) * J)
                eng.scalar_tensor_tensor(F[:, s2], K1[:, s2], MAGIC,
                                         Q[:, s2], ALU.subtract,
                                         ALU.subtract)

            def cos_t(i):
                sl = slice(i * J, (i + 1) * J)
                nc.scalar.activation(CT[:, sl], F[:, (2 * i + 1) * J:
                                                    (2 * i + 2) * J],
                                     SIN, scale=NEG_2PI)

            def sin_t(i, m):
                for b0, bs, cnt, sgn in S_RUNS[m]:
                    dst = _ap_with_dims(ST[:], (SBASE[m] + b0) * J,
                                        [[bs * J, cnt], [1, J]])
                    src = _ap_with_dims(F[:], 2 * i * J, [[0, cnt], [1, J]])
                    nc.scalar.activation(dst, src, SIN,
                                         scale=NEG_2PI * sgn)

            def tables_direct(i, m):
                # no-range-reduction path: C and S straight from posf
                nc.scalar.activation(CT[:, i * J:(i + 1) * J], POSF[:], SIN,
                                     scale=-fc[i], bias=HPI[:])
                for b0, bs, cnt, sgn in S_RUNS[m]:
                    dst = _ap_with_dims(ST[:], (SBASE[m] + b0) * J,
                                        [[bs * J, cnt], [1, J]])
                    src = _ap_with_dims(POSF[:], 0, [[0, cnt], [1, J]])
                    nc.scalar.activation(dst, src, SIN, scale=fc[i] * sgn)

            # stage 0 and 1 tables up front (direct planes are pure ScalarE;
            # a non-direct stage here falls back to the q-phase chain with
            # its F-step placed in the DVE's pre-rotation idle window)
            for s in (0, 1):
                si, sm = STAGES[s]
                if direct[si]:
                    tables_direct(si, sm)
                else:
                    affine_c(si)
                    affine_s(si)
                    fstep(si, 0, 2, nc.vector)
                    cos_t(si)
                    sin_t(si, sm)

            for k, (i, m) in enumerate(STAGES):
                ops = ROT[m]
                for sub in ops["tsubs"]:
                    xr = _ap_with_dims(X[:], sub["xoff"] * J,
                                       [[s * J, c] for s, c in sub["xdims"]]
                                       + [[1, J]])
                    tw = _ap_with_dims(T[:], sub["toff"] * J,
                                       [[s * J, c] for s, c in sub["tdims"]]
                                       + [[1, J]])
                    cb = _ap_with_dims(CT[:], i * J,
                                       [[0, c] for (_, c) in sub["tdims"]]
                                       + [[1, J]])
                    nc.vector.tensor_mul(tw, xr, cb)
                for sub in ops["usubs"]:
                    xr = _ap_with_dims(X[:], sub["xoff"] * J,
                                       [[s * J, c] for s, c in sub["xdims"]]
                                       + [[1, J]])
                    uw = _ap_with_dims(U[:], sub["toff"] * J,
                                       [[s * J, c] for s, c in sub["tdims"]]
                                       + [[1, J]])
                    sb = _ap_with_dims(ST[:], (SBASE[m] + sub["soff"]) * J,
                                       [[s * J, c] for s, c in
                                        zip(sub["ssteps"],
                                            [c for (_, c) in sub["tdims"]])]
                                       + [[1, J]])
                    nc.vector.tensor_mul(uw, xr, sb)

                if k + 1 < len(STAGES) and k >= 1:
                    ni, nm = STAGES[k + 1]
                    if direct[ni]:
                        tables_direct(ni, nm)
                    else:
                        fstep(ni, 0, 2, nc.vector)
                        cos_t(ni)
                        sin_t(ni, nm)
                if k + 2 < len(STAGES) and not direct[STAGES[k + 2][0]]:
                    affine_c(STAGES[k + 2][0])
                    affine_s(STAGES[k + 2][0])

                final = k == len(STAGES) - 1
                for sub in ops["asubs"]:
                    xw = _ap_with_dims(X[:], sub["xoff"] * J,
                                       [[s * J, c] for s, c in sub["xdims"]]
                                       + [[1, J]])
                    tv = _ap_with_dims(T[:], sub["toff"] * J,
                                       [[s * J, c] for s, c in sub["tdims"]]
                                       + [[1, J]])
                    uv = _ap_with_dims(U[:], sub["toff"] * J,
                                       [[s * J, c] for s, c in sub["tdims"]]
                                       + [[1, J]])
                    nc.vector.tensor_add(xw, tv, uv)
                    if final:
                        # this quartet is done; comp pairs are contiguous
                        # 2J runs
                        dsts = _ap_with_dims(out_d[:], sub["xoff"] * J,
                                             [[4 * J, 2], [1, 2 * J]])
                        srcs = _ap_with_dims(X[:], sub["xoff"] * J,
                                             [[4 * J, 2], [1, 2 * J]])
                        nc.sync.dma_start(dsts, srcs)

                if m == 9:   # comps 7,8 final
                    nc.sync.dma_start(out_d[:, 7 * J:9 * J], X[:, 7 * J:9 * J])
                if m == 5:   # comps 3,4,11,12 final (pairs comp-contiguous)
                    dsts = _ap_with_dims(out_d[:], 3 * J,
                                         [[8 * J, 2], [1, 2 * J]])
                    srcs = _ap_with_dims(X[:], 3 * J,
                                         [[8 * J, 2], [1, 2 * J]])
                    nc.sync.dma_start(dsts, srcs)

    nc.compile()
    return nc


_PROGRAM_CACHE = {}


def _get_program(freqs, coefs):
    key = (tuple(freqs), tuple(coefs))
    if key not in _PROGRAM_CACHE:
        _PROGRAM_CACHE[key] = _build_program(freqs, coefs)
    return _PROGRAM_CACHE[key]


def _core_inputs(x, pos_i, g):
    """Per-core comp-major fp16 x and fp32 posf."""
    xg = np.ascontiguousarray(
        x[g * ROWS_PER_CORE:(g + 1) * ROWS_PER_CORE]).reshape(P, J, MV)
    xg = np.ascontiguousarray(xg.transpose(0, 2, 1)).astype(np.float16)
    pg = pos_i[g * ROWS_PER_CORE:(g + 1) * ROWS_PER_CORE].reshape(P, J)
    return {"x": xg.reshape(P, MV * J),
            "posf": np.ascontiguousarray(pg).astype(np.float32)}


def kernel(x, pos, bx, by, bz, bw, theta, cayley, biv_mask, scalar_mask):
    x = np.asarray(x, dtype=np.float32)
    pos = np.asarray(pos)
    theta = np.asarray(theta, dtype=np.float32)
    cayley = np.asarray(cayley, dtype=np.float32)

    assert x.shape == (B, L, MV) and pos.shape == (B, L)

    coefs = [float(np.asarray(c, dtype=np.float32).reshape(MV)[b])
             for c, b in zip((bx, by, bz, bw), PLANE_BLADES)]
    freqs = [float(theta.reshape(MAX_LEN, 4)[1, i]) for i in range(4)]
    th_check = np.arange(MAX_LEN, dtype=np.float32)[:, None] * \
        np.asarray(freqs, dtype=np.float32)[None, :]
    assert np.array_equal(th_check, theta.reshape(MAX_LEN, 4)), \
        "theta table is not linear in position; kernel assumption violated"

    _verify_rot(cayley)

    nc = _get_program(freqs, coefs)

    pos_i = np.clip(pos, 0, MAX_LEN - 1).astype(np.int32)
    in_maps = [_core_inputs(x, pos_i, g) for g in range(NCORES)]

    res = run_bass_kernel_spmd(nc, in_maps, core_ids=list(range(NCORES)))
    out = np.empty((B, L, MV), dtype=np.float32)
    for g in range(NCORES):
        og = res.results[g]["out"].reshape(P, MV, J).astype(np.float32)
        out[g * ROWS_PER_CORE:(g + 1) * ROWS_PER_CORE] = \
            og.transpose(0, 2, 1).reshape(ROWS_PER_CORE, L, MV)
    # scalar / pseudoscalar components are invariant under the sandwich
    out[..., 0] = x[..., 0]
    out[..., 15] = x[..., 15]
    return out
